# revision 10
# baseline (speedup 1.0000x reference)
"""GCN VGAE encoder (nn_Encoder_25065429139538) on 8 Trainium2 NeuronCores.

Strategy (sharding_hint: shard nodes across cores, partition edges by dst,
replicate weights):
  - Nodes padded to 100352 = 8 x 12544; core d owns dst rows [d*SH, (d+1)*SH).
  - Per-edge GCN normalization (incl. self-loops) folded into one per-edge
    scalar on the host; duplicate (src,dst) merged.
  - Tables (the per-layer gather source) are fp16 [NPAD, 64] viewed as
    pair-packed [NPAD/2, 128] rows of 256B (dma_gather elem floor). Edges are
    parity-sorted inside each chunk: slots 0-63 hold even-src edges (cols
    0:64 of the gathered pair-row), slots 64-127 odd-src (cols 64:128), so
    the half-select is two static DVE slices.
  - Aggregation per dst 128-block via one-hot matmul accumulating in PSUM
    [HID, 128]: lhsT = scaled gathered rows (fp16), rhs = one-hot built from
    dst-local ids by DVE is_equal. Self-loops ride as one extra matmul with
    rhs = identity and lhsT = previous-layer rows scaled by dinv^2.
  - Bias (+ReLU for layer 1) applied by ScalarE per-partition bias during
    PSUM->SBUF eviction; the next layer's projection (W2 / [Wmu|Wls]) is
    folded BEFORE the AllGather (aggregation commutes with projection), so
    each pass is gather -> scale -> one-hot matmul -> bias -> project.
  - All gather metadata (indices, dst-locals, norms) is host-transposed to
    partition-major layout and loaded once into SBUF (no per-group DMAs).
  - AllGather in fp16 between layers rebuilds the full gather table.
"""

import math

import numpy as np

import concourse.bass as bass
import concourse.bacc as bacc
import concourse.mybir as mybir
import concourse.tile as tile
from concourse.bass_utils import run_bass_kernel_spmd
from concourse.library_config import mlp

# ---- problem constants (hardcoded per contract) ----
N = 100000
FIN, HID, OUT = 128, 64, 32
NCORES = 8

# ---- layout constants ----
SH = 12544             # rows per core (100352 / 8)
NPAD = SH * NCORES     # 100352
NBLK = SH // 128       # 98 dst blocks per core
NSUB = 2               # src subtables (int16 pair-row indices < 25088)
SUB = NPAD // NSUB     # 50176 nodes per subtable
SUBP = SUB // 2        # 25088 pair-rows per subtable
SLOTS = 1024           # gather slots per dma_gather instruction
CPG = SLOTS // 128     # chunks per gather group = 16
HALF = 64              # parity slots per chunk


def _wrap_idx(slots_i16):
    """[G*SLOTS] int16 -> [G, 128, SLOTS//16]: slot i at [i%16 (+16m), i//16]."""
    g = slots_i16.reshape(-1, SLOTS // 16, 16)          # [G, S/16, 16]
    g = np.swapaxes(g, 1, 2)                            # [G, 16, S/16]
    return np.tile(g, (1, 8, 1)).astype(np.int16)       # [G, 128, S/16]


def _prep(x, edge_index, edge_weight):
    """Host-side edge partitioning. Returns (structure, per-core arrays)."""
    src = np.asarray(edge_index[0], dtype=np.int64)
    dst = np.asarray(edge_index[1], dtype=np.int64)
    ew = np.asarray(edge_weight, dtype=np.float32)

    deg_w = np.zeros(N, np.float32)
    np.add.at(deg_w, dst, ew)
    deg_w += 1.0  # self-loop weight
    deg_1 = (np.bincount(dst, minlength=N) + 1).astype(np.float32)
    dinv_w = 1.0 / np.sqrt(deg_w)
    dinv_1 = 1.0 / np.sqrt(deg_1)

    nw = dinv_w[src] * ew * dinv_w[dst]
    n1 = dinv_1[src] * dinv_1[dst]

    # merge duplicate (src, dst) pairs (self-loops handled separately)
    key = src * NPAD + dst
    ukey, inv = np.unique(key, return_inverse=True)
    unw = np.zeros(len(ukey), np.float32)
    un1 = np.zeros(len(ukey), np.float32)
    np.add.at(unw, inv, nw)
    np.add.at(un1, inv, n1)
    usrc = ukey // NPAD
    udst = ukey % NPAD

    core = udst // SH
    t_all = (udst % SH) // 128
    dloc_all = (udst % SH) % 128
    s_all = usrc // SUB
    prow_all = (usrc % SUB) // 2
    par_all = usrc % 2

    # per (core, t, s, par) counts -> shared chunk structure K_ts
    cell = ((core * NBLK + t_all) * NSUB + s_all) * 2 + par_all
    cnt = np.bincount(cell, minlength=NCORES * NBLK * NSUB * 2).reshape(
        NCORES, NBLK, NSUB, 2
    )
    K_ts = np.maximum(1, np.ceil(cnt.max(axis=0).max(axis=-1) / HALF).astype(np.int64))
    C_s = K_ts.sum(axis=0)                      # chunks per s-stream
    G_s = [int(math.ceil(int(c) / CPG)) for c in C_s]
    base_pos = np.zeros((NBLK, NSUB), np.int64)  # chunk stream position of (t,s)
    for s in range(NSUB):
        base_pos[:, s] = np.cumsum(np.concatenate([[0], K_ts[:-1, s]]))

    per_core = []
    # order by (core, t, s, par, prow): prow-sorted within each parity run
    order = np.lexsort((prow_all, par_all, s_all, t_all, core))
    oprow, opar, os_, ot, odloc, onw, on1, ocore = (
        prow_all[order], par_all[order], s_all[order], t_all[order],
        dloc_all[order], unw[order], un1[order], core[order],
    )
    cstart = np.searchsorted(ocore, np.arange(NCORES + 1))
    for d in range(NCORES):
        lo, hi = cstart[d], cstart[d + 1]
        dt, ds, dp = ot[lo:hi], os_[lo:hi], opar[lo:hi]
        dpr, ddl = oprow[lo:hi], odloc[lo:hi]
        dnw, dn1 = onw[lo:hi], on1[lo:hi]
        cell_d = (dt * NSUB + ds) * 2 + dp
        cello = np.searchsorted(cell_d, np.arange(NBLK * NSUB * 2 + 1))
        idxs, dlocs, nws, n1s = [], [], [], []
        for s in range(NSUB):
            nslot = G_s[s] * SLOTS
            sl = np.zeros(nslot, np.int64)
            dl = np.zeros(nslot, np.float32)
            wv = np.zeros(nslot, np.float32)
            v1 = np.zeros(nslot, np.float32)
            for t in range(NBLK):
                p0 = base_pos[t, s]
                for par in range(2):
                    c = (t * NSUB + s) * 2 + par
                    a, b = cello[c], cello[c + 1]
                    n = b - a
                    # fill slots chunk by chunk: chunk k slots
                    # [(p0+k)*128 + par*64, +64)
                    for k in range(int(math.ceil(n / HALF)) or 0):
                        u, v = k * HALF, min((k + 1) * HALF, n)
                        base = (p0 + k) * 128 + par * HALF
                        sl[base:base + v - u] = dpr[a + u:a + v]
                        dl[base:base + v - u] = ddl[a + u:a + v]
                        wv[base:base + v - u] = dnw[a + u:a + v]
                        v1[base:base + v - u] = dn1[a + u:a + v]
            idxs.append(_wrap_idx(sl.astype(np.int16)))
            dlocs.append(np.ascontiguousarray(
                dl.reshape(G_s[s], CPG, 128).transpose(2, 0, 1)
            ).astype(np.float16).reshape(128, -1))
            nws.append(np.ascontiguousarray(
                wv.reshape(G_s[s], CPG, 128).transpose(2, 0, 1)
            ).astype(np.float16).reshape(128, -1))
            n1s.append(np.ascontiguousarray(
                v1.reshape(G_s[s], CPG, 128).transpose(2, 0, 1)
            ).astype(np.float16).reshape(128, -1))
        # self-loop norms [128, NBLK]: [p, t] = dinv^2 of row d*SH + t*128 + p
        v_glob = np.arange(d * SH, (d + 1) * SH, dtype=np.int64)
        real = v_glob < N
        sw = np.zeros(SH, np.float32)
        s1 = np.zeros(SH, np.float32)
        sw[real] = (dinv_w * dinv_w)[v_glob[real]]
        s1[real] = (dinv_1 * dinv_1)[v_glob[real]]
        snw = np.ascontiguousarray(sw.reshape(NBLK, 128).T).astype(np.float16)
        sn1 = np.ascontiguousarray(s1.reshape(NBLK, 128).T).astype(np.float16)
        per_core.append((idxs, dlocs, nws, n1s, snw, sn1))

    return K_ts, G_s, base_pos, per_core


def _build(K_ts, G_s, base_pos):
    f32 = mybir.dt.float32
    f16 = mybir.dt.float16
    i16 = mybir.dt.int16
    nc = bacc.Bacc(None, target_bir_lowering=False, num_swdge_queues=4,
                   num_devices=NCORES)

    xT_d = nc.dram_tensor("xT", [FIN, SH], f32, kind="ExternalInput")
    idx_d = [nc.dram_tensor(f"idx{s}", [G_s[s], 128, SLOTS // 16], i16,
                            kind="ExternalInput") for s in range(NSUB)]
    dloc_d = [nc.dram_tensor(f"dloc{s}", [128, G_s[s] * CPG], f16,
                             kind="ExternalInput") for s in range(NSUB)]
    nw_d = [nc.dram_tensor(f"nw{s}", [128, G_s[s] * CPG], f16,
                           kind="ExternalInput") for s in range(NSUB)]
    n1_d = [nc.dram_tensor(f"n1{s}", [128, G_s[s] * CPG], f16,
                           kind="ExternalInput") for s in range(NSUB)]
    snw_d = nc.dram_tensor("snw", [128, NBLK], f16, kind="ExternalInput")
    sn1_d = nc.dram_tensor("sn1", [128, NBLK], f16, kind="ExternalInput")
    w1_d = nc.dram_tensor("W1", [FIN, HID], f32, kind="ExternalInput")
    w2_d = nc.dram_tensor("W2", [HID, HID], f16, kind="ExternalInput")
    wml_d = nc.dram_tensor("Wml", [HID, 2 * OUT], f16, kind="ExternalInput")
    b1_d = nc.dram_tensor("b1", [HID, 1], f32, kind="ExternalInput")
    b2_d = nc.dram_tensor("b2", [HID, 1], f32, kind="ExternalInput")
    bml_d = nc.dram_tensor("bml", [2 * OUT, 1], f32, kind="ExternalInput")
    iota_d = nc.dram_tensor("iota", [128, CPG * 128], f16, kind="ExternalInput")
    id128_d = nc.dram_tensor("id128", [128, 128], f16, kind="ExternalInput")
    id64_d = nc.dram_tensor("id64", [HID, HID], f16, kind="ExternalInput")
    out_d = nc.dram_tensor("out", [SH, 2 * OUT], f32, kind="ExternalOutput")

    ag_in = [nc.dram_tensor(f"ag_in{i}", [SH, HID], f16) for i in range(3)]
    tables = [nc.dram_tensor(f"table{i}", [NPAD, HID], f16, addr_space="Shared")
              for i in range(3)]

    with tile.TileContext(nc) as tc:
        with (
            tc.tile_pool(name="const", bufs=1) as kpool,
            tc.tile_pool(name="meta", bufs=1) as mpool,
            tc.tile_pool(name="idx", bufs=8) as ipool,
            tc.tile_pool(name="g", bufs=2) as gpool,
            tc.tile_pool(name="b", bufs=2) as bpool,
            tc.tile_pool(name="gh", bufs=2) as ghpool,
            tc.tile_pool(name="rows", bufs=2) as rpool,
            tc.tile_pool(name="selfh", bufs=1) as selfpool,
            tc.tile_pool(name="hcol", bufs=1) as hpool,
            tc.tile_pool(name="stf", bufs=1) as stpool,
            tc.tile_pool(name="xt", bufs=3) as xtpool,
            tc.tile_pool(name="pagg", bufs=4, space="PSUM") as pagg,
            tc.tile_pool(name="pproj", bufs=2, space="PSUM") as pproj,
            tc.tile_pool(name="ptr", bufs=2, space="PSUM") as ptr,
        ):
            nc.gpsimd.load_library(mlp)

            def load_const(pool, dram, shape, dt):
                t = pool.tile(shape, dt, name=dram.name + "_t")
                nc.sync.dma_start(t[:], dram[:])
                return t

            iota_t = load_const(kpool, iota_d, [128, CPG * 128], f16)
            id128_t = load_const(kpool, id128_d, [128, 128], f16)
            id64_t = load_const(kpool, id64_d, [HID, HID], f16)
            w1_t = load_const(kpool, w1_d, [FIN, HID], f32)
            w2_t = load_const(kpool, w2_d, [HID, HID], f16)
            wml_t = load_const(kpool, wml_d, [HID, 2 * OUT], f16)
            b1_t = load_const(kpool, b1_d, [HID, 1], f32)
            b2_t = load_const(kpool, b2_d, [HID, 1], f32)
            bml_t = load_const(kpool, bml_d, [2 * OUT, 1], f32)
            snw_t = load_const(kpool, snw_d, [128, NBLK], f16)
            sn1_t = load_const(kpool, sn1_d, [128, NBLK], f16)

            dloc_t, nw_t, n1_t = [], [], []
            for s in range(NSUB):
                dt_ = mpool.tile([128, G_s[s] * CPG], f16, tag=f"dl{s}")
                nc.sync.dma_start(dt_[:], dloc_d[s][:])
                dloc_t.append(dt_)
                wt = mpool.tile([128, G_s[s] * CPG], f16, tag=f"nw{s}")
                nc.sync.dma_start(wt[:], nw_d[s][:])
                nw_t.append(wt)
                ot = mpool.tile([128, G_s[s] * CPG], f16, tag=f"n1{s}")
                nc.sync.dma_start(ot[:], n1_d[s][:])
                n1_t.append(ot)

            def rows_tile():
                return rpool.tile([128, NBLK, HID], f16, tag="rows",
                                  name="rows")

            # ---- table0 = x @ W1 (fp16 rows) ----
            rows_prev = rows_tile()
            for t in range(NBLK):
                xt = xtpool.tile([128, 128], f32, tag="xt")
                nc.sync.dma_start(xt[:], xT_d[:, t * 128:(t + 1) * 128])
                pp = pproj.tile([128, HID], f32, tag="pproj")
                nc.tensor.matmul(pp[:], lhsT=xt[:], rhs=w1_t[:],
                                 start=True, stop=True)
                nc.scalar.activation(rows_prev[:, t, :], pp[:],
                                     mybir.ActivationFunctionType.Copy)
            nc.sync.dma_start(
                ag_in[0][:].rearrange("(t p) f -> p t f", p=128), rows_prev[:])

            def allgather(i):
                nc.gpsimd.collective_compute(
                    "AllGather", mybir.AluOpType.bypass,
                    replica_groups=[list(range(NCORES))],
                    ins=[ag_in[i][:]], outs=[tables[i][:]],
                )

            allgather(0)

            gq = [0]

            def gather_pass(tbl_i, norms, snorm_t, func, bias_t, rows_prev,
                            w_next):
                """One aggregation pass over tables[tbl_i].

                Returns rows_next (projected by w_next) or hcol (w_next None,
                final pass -> mu/ls columns)."""
                selfh = selfpool.tile([128, NBLK, HID], f16, tag="selfh")
                nc.vector.tensor_tensor(
                    out=selfh[:], in0=rows_prev[:],
                    in1=snorm_t[:].to_broadcast([128, NBLK, HID]),
                    op=mybir.AluOpType.mult,
                )
                hcol = hpool.tile([HID, NBLK, 128], f16, tag="hcol")
                rows_next = rows_tile() if w_next is not None else None
                cur = {}

                def ensure(s, g):
                    if s in cur and cur[s][0] == g:
                        return cur[s][1]
                    it = ipool.tile([128, SLOTS // 16], i16, tag="idx")
                    nc.sync.dma_start(it[:], idx_d[s][g])
                    gt = gpool.tile([128, CPG, 128], f16, tag=f"g{s}")
                    nc.gpsimd.dma_gather(
                        gt[:],
                        tables[tbl_i][s * SUB:(s + 1) * SUB, :].rearrange(
                            "(r two) f -> r (two f)", two=2),
                        it[:],
                        SLOTS, SLOTS, 128, queue_num=gq[0] % 4,
                    )
                    gq[0] += 1
                    bt = bpool.tile([128, CPG, 128], f16, tag=f"b{s}")
                    nc.vector.tensor_tensor(
                        out=bt[:],
                        in0=iota_t[:].rearrange("p (j v) -> p j v", j=CPG),
                        in1=dloc_t[s][:, g * CPG:(g + 1) * CPG].to_broadcast(
                            [128, CPG, 128]),
                        op=mybir.AluOpType.is_equal,
                    )
                    gh = ghpool.tile([128, CPG, HID], f16, tag=f"gh{s}")
                    nt = norms[s]
                    nc.vector.tensor_tensor(
                        out=gh[:HALF],
                        in0=gt[:HALF, :, 0:HID],
                        in1=nt[:HALF, g * CPG:(g + 1) * CPG].to_broadcast(
                            [HALF, CPG, HID]),
                        op=mybir.AluOpType.mult,
                    )
                    nc.vector.tensor_tensor(
                        out=gh[HALF:],
                        in0=gt[HALF:, :, HID:2 * HID],
                        in1=nt[HALF:, g * CPG:(g + 1) * CPG].to_broadcast(
                            [HALF, CPG, HID]),
                        op=mybir.AluOpType.mult,
                    )
                    cur[s] = (g, (gh, bt))
                    return gh, bt

                for t in range(NBLK):
                    ps = pagg.tile([HID, 128], f32, tag="pagg")
                    nc.tensor.matmul(ps[:], lhsT=selfh[:, t, :], rhs=id128_t[:],
                                     start=True, stop=False)
                    nchunks = int(K_ts[t].sum())
                    ci = 0
                    for s in range(NSUB):
                        for k in range(int(K_ts[t, s])):
                            pos = int(base_pos[t, s]) + k
                            g, j = divmod(pos, CPG)
                            gh, bt = ensure(s, g)
                            ci += 1
                            nc.tensor.matmul(
                                ps[:], lhsT=gh[:, j, :], rhs=bt[:, j, :],
                                start=False, stop=(ci == nchunks),
                            )
                    nc.scalar.activation(hcol[:, t, :], ps[:], func,
                                         bias=bias_t[:])
                    if w_next is not None:
                        pp = pproj.tile([128, HID], f32, tag="pproj")
                        nc.tensor.matmul(pp[:], lhsT=hcol[:, t, :],
                                         rhs=w_next[:], start=True, stop=True)
                        nc.scalar.activation(rows_next[:, t, :], pp[:],
                                             mybir.ActivationFunctionType.Copy)
                return rows_next if w_next is not None else hcol

            relu = mybir.ActivationFunctionType.Relu
            copy = mybir.ActivationFunctionType.Identity

            # ---- layer 1: aggregate table0, relu(+b1), project W2 ----
            rows1 = gather_pass(0, nw_t, snw_t, relu, b1_t, rows_prev, w2_t)
            nc.sync.dma_start(
                ag_in[1][:].rearrange("(t p) f -> p t f", p=128), rows1[:])
            allgather(1)

            # ---- layer 2: aggregate table1, +b2, project [Wmu|Wls] ----
            rows2 = gather_pass(1, nw_t, snw_t, copy, b2_t, rows1, wml_t)
            nc.sync.dma_start(
                ag_in[2][:].rearrange("(t p) f -> p t f", p=128), rows2[:])
            allgather(2)

            # ---- layer 3: aggregate table2, +[bmu|bls] -> mu/ls columns ----
            mlcol = gather_pass(2, n1_t, sn1_t, copy, bml_t, rows2, None)

            # transpose [64, 128] -> [128, 64] per block and store
            stf = stpool.tile([128, NBLK, 2 * OUT], f32, tag="stf")
            for t in range(NBLK):
                pt = ptr.tile([128, 2 * OUT], f32, tag="ptr")
                nc.tensor.matmul(pt[:], lhsT=mlcol[:, t, :], rhs=id64_t[:],
                                 start=True, stop=True)
                nc.scalar.activation(stf[:, t, :], pt[:], copy)
            nc.sync.dma_start(
                out_d[:].rearrange("(t p) f -> p t f", p=128), stf[:])

    # Tile round-robins Pool-DMA completion sems over 8 DMASW lanes without
    # queue awareness, but each sem is hardware-locked to the first SWDGE
    # queue that increments it. Rewrite each gather's queue to lane % 4 so
    # every lane's sem is only ever incremented from one queue.
    for fn in nc.m.functions:
        for blk in fn.blocks:
            for ins in blk.instructions:
                if isinstance(ins, mybir.InstDMAGatherAnt) and ins.sync_info:
                    for u in ins.sync_info.on_update:
                        name = getattr(u, "ant_name", "") or ""
                        if name.startswith("DMASW"):
                            ins.queue_num = int(name[5:].split("_")[0]) % 4
                            break

    nc.compile()
    return nc


def _run(inputs, trace=False):
    x = np.asarray(inputs["x"], np.float32)
    K_ts, G_s, base_pos, per_core = _prep(
        x, np.asarray(inputs["edge_index"]), np.asarray(inputs["edge_weight"])
    )
    nc = _build(K_ts, G_s, base_pos)

    x_pad = np.zeros((NPAD, FIN), np.float32)
    x_pad[:N] = x
    iota = np.tile(np.arange(128, dtype=np.float16)[None, :], (128, CPG))
    shared = {
        "W1": np.asarray(inputs["W1"], np.float32),
        "W2": np.asarray(inputs["W2"], np.float16),
        "Wml": np.concatenate(
            [np.asarray(inputs["Wmu"], np.float16),
             np.asarray(inputs["Wls"], np.float16)], axis=1),
        "b1": np.asarray(inputs["b1"], np.float32).reshape(HID, 1),
        "b2": np.asarray(inputs["b2"], np.float32).reshape(HID, 1),
        "bml": np.concatenate(
            [np.asarray(inputs["bmu"], np.float32),
             np.asarray(inputs["bls"], np.float32)]).reshape(2 * OUT, 1),
        "iota": iota.reshape(128, CPG * 128),
        "id128": np.eye(128, dtype=np.float16),
        "id64": np.eye(HID, dtype=np.float16),
    }
    in_maps = []
    for d in range(NCORES):
        idxs, dlocs, nws, n1s, snw, sn1 = per_core[d]
        m = dict(shared)
        m["xT"] = np.ascontiguousarray(x_pad[d * SH:(d + 1) * SH].T)
        m["snw"] = snw
        m["sn1"] = sn1
        for s in range(NSUB):
            m[f"idx{s}"] = idxs[s]
            m[f"dloc{s}"] = dlocs[s]
            m[f"nw{s}"] = nws[s]
            m[f"n1{s}"] = n1s[s]
        in_maps.append(m)

    res = run_bass_kernel_spmd(nc, in_maps, core_ids=list(range(NCORES)), trace=trace)
    full = np.concatenate([res.results[d]["out"] for d in range(NCORES)], axis=0)
    mu = full[:N, :OUT].copy()
    logstd = full[:N, OUT:].copy()
    return (mu, logstd), res


def kernel(**inputs):
    (mu, logstd), _ = _run(inputs, trace=False)
    return mu, logstd


# revision 11
# speedup vs baseline: 1.2737x; 1.2737x over previous
"""GCN VGAE encoder (nn_Encoder_25065429139538) on 8 Trainium2 NeuronCores.

Strategy (sharding_hint: shard nodes across cores, partition edges by dst,
replicate weights):
  - Nodes padded to 100352 = 8 x 12544; core d owns dst rows [d*SH, (d+1)*SH).
  - Per-edge GCN normalization (incl. self-loops) folded into one per-edge
    scalar on the host; duplicate (src,dst) merged.
  - Tables (the per-layer gather source) are fp16 [NPAD, 64] viewed as
    pair-packed [NPAD/2, 128] rows of 256B (dma_gather elem floor). Edges are
    parity-sorted inside each chunk: slots 0-63 hold even-src edges (cols
    0:64 of the gathered pair-row), slots 64-127 odd-src (cols 64:128), so
    the half-select is two static DVE slices.
  - Aggregation per dst 128-block via one-hot matmul accumulating in PSUM
    [HID, 128]: lhsT = scaled gathered rows (fp16), rhs = one-hot built from
    dst-local ids by DVE is_equal. Self-loops ride as one extra matmul with
    rhs = identity and lhsT = previous-layer rows scaled by dinv^2.
  - Bias (+ReLU for layer 1) applied by ScalarE per-partition bias during
    PSUM->SBUF eviction; the next layer's projection (W2 / [Wmu|Wls]) is
    folded BEFORE the AllGather (aggregation commutes with projection), so
    each pass is gather -> scale -> one-hot matmul -> bias -> project.
  - All gather metadata (indices, dst-locals, norms) is host-transposed to
    partition-major layout and loaded once into SBUF (no per-group DMAs).
  - AllGather in fp16 between layers rebuilds the full gather table.
"""

import math

import numpy as np

import concourse.bass as bass
import concourse.bacc as bacc
import concourse.mybir as mybir
import concourse.tile as tile
from concourse.bass_utils import run_bass_kernel_spmd
from concourse.library_config import mlp

# ---- problem constants (hardcoded per contract) ----
N = 100000
FIN, HID, OUT = 128, 64, 32
NCORES = 8

# ---- layout constants ----
SH = 12544             # rows per core (100352 / 8)
NPAD = SH * NCORES     # 100352
NBLK = SH // 128       # 98 dst blocks per core
NSUB = 2               # src subtables (int16 pair-row indices < 25088)
SUB = NPAD // NSUB     # 50176 nodes per subtable
SUBP = SUB // 2        # 25088 pair-rows per subtable
SLOTS = 1024           # gather slots per dma_gather instruction
CPG = SLOTS // 128     # chunks per gather group = 16
HALF = 64              # parity slots per chunk


def _wrap_idx(slots_i16):
    """[G*SLOTS] int16 -> [G, 128, SLOTS//16]: slot i at [i%16 (+16m), i//16]."""
    g = slots_i16.reshape(-1, SLOTS // 16, 16)          # [G, S/16, 16]
    g = np.swapaxes(g, 1, 2)                            # [G, 16, S/16]
    return np.tile(g, (1, 8, 1)).astype(np.int16)       # [G, 128, S/16]


def _prep(x, edge_index, edge_weight):
    """Host-side edge partitioning. Returns (structure, per-core arrays)."""
    src = np.asarray(edge_index[0], dtype=np.int64)
    dst = np.asarray(edge_index[1], dtype=np.int64)
    ew = np.asarray(edge_weight, dtype=np.float32)

    deg_w = np.zeros(N, np.float32)
    np.add.at(deg_w, dst, ew)
    deg_w += 1.0  # self-loop weight
    deg_1 = (np.bincount(dst, minlength=N) + 1).astype(np.float32)
    dinv_w = 1.0 / np.sqrt(deg_w)
    dinv_1 = 1.0 / np.sqrt(deg_1)

    nw = dinv_w[src] * ew * dinv_w[dst]
    n1 = dinv_1[src] * dinv_1[dst]

    # merge duplicate (src, dst) pairs (self-loops handled separately)
    key = src * NPAD + dst
    ukey, inv = np.unique(key, return_inverse=True)
    unw = np.zeros(len(ukey), np.float32)
    un1 = np.zeros(len(ukey), np.float32)
    np.add.at(unw, inv, nw)
    np.add.at(un1, inv, n1)
    usrc = ukey // NPAD
    udst = ukey % NPAD

    core = udst // SH
    t_all = (udst % SH) // 128
    dloc_all = (udst % SH) % 128
    s_all = usrc // SUB
    prow_all = (usrc % SUB) // 2
    par_all = usrc % 2

    # per (core, t, s, par) counts -> shared chunk structure K_ts
    cell = ((core * NBLK + t_all) * NSUB + s_all) * 2 + par_all
    cnt = np.bincount(cell, minlength=NCORES * NBLK * NSUB * 2).reshape(
        NCORES, NBLK, NSUB, 2
    )
    K_ts = np.maximum(1, np.ceil(cnt.max(axis=0).max(axis=-1) / HALF).astype(np.int64))
    C_s = K_ts.sum(axis=0)                      # chunks per s-stream
    G_s = [int(math.ceil(int(c) / CPG)) for c in C_s]
    base_pos = np.zeros((NBLK, NSUB), np.int64)  # chunk stream position of (t,s)
    for s in range(NSUB):
        base_pos[:, s] = np.cumsum(np.concatenate([[0], K_ts[:-1, s]]))

    per_core = []
    # order by (core, t, s, par, prow): prow-sorted within each parity run
    order = np.lexsort((prow_all, par_all, s_all, t_all, core))
    oprow, opar, os_, ot, odloc, onw, on1, ocore = (
        prow_all[order], par_all[order], s_all[order], t_all[order],
        dloc_all[order], unw[order], un1[order], core[order],
    )
    cstart = np.searchsorted(ocore, np.arange(NCORES + 1))
    for d in range(NCORES):
        lo, hi = cstart[d], cstart[d + 1]
        dt, ds, dp = ot[lo:hi], os_[lo:hi], opar[lo:hi]
        dpr, ddl = oprow[lo:hi], odloc[lo:hi]
        dnw, dn1 = onw[lo:hi], on1[lo:hi]
        cell_d = (dt * NSUB + ds) * 2 + dp
        cello = np.searchsorted(cell_d, np.arange(NBLK * NSUB * 2 + 1))
        idxs, dlocs, nws, n1s = [], [], [], []
        for s in range(NSUB):
            nslot = G_s[s] * SLOTS
            sl = np.zeros(nslot, np.int64)
            dl = np.zeros(nslot, np.float32)
            wv = np.zeros(nslot, np.float32)
            v1 = np.zeros(nslot, np.float32)
            for t in range(NBLK):
                p0 = base_pos[t, s]
                for par in range(2):
                    c = (t * NSUB + s) * 2 + par
                    a, b = cello[c], cello[c + 1]
                    n = b - a
                    # fill slots chunk by chunk: chunk k slots
                    # [(p0+k)*128 + par*64, +64)
                    for k in range(int(math.ceil(n / HALF)) or 0):
                        u, v = k * HALF, min((k + 1) * HALF, n)
                        base = (p0 + k) * 128 + par * HALF
                        sl[base:base + v - u] = dpr[a + u:a + v]
                        dl[base:base + v - u] = ddl[a + u:a + v]
                        wv[base:base + v - u] = dnw[a + u:a + v]
                        v1[base:base + v - u] = dn1[a + u:a + v]
            idxs.append(_wrap_idx(sl.astype(np.int16)))
            dlocs.append(np.ascontiguousarray(
                dl.reshape(G_s[s], CPG, 128).transpose(2, 0, 1)
            ).astype(np.float16).reshape(128, -1))
            nws.append(np.ascontiguousarray(
                wv.reshape(G_s[s], CPG, 128).transpose(2, 0, 1)
            ).astype(np.float16).reshape(128, -1))
            n1s.append(np.ascontiguousarray(
                v1.reshape(G_s[s], CPG, 128).transpose(2, 0, 1)
            ).astype(np.float16).reshape(128, -1))
        # self-loop norms [128, NBLK]: [p, t] = dinv^2 of row d*SH + t*128 + p
        v_glob = np.arange(d * SH, (d + 1) * SH, dtype=np.int64)
        real = v_glob < N
        sw = np.zeros(SH, np.float32)
        s1 = np.zeros(SH, np.float32)
        sw[real] = (dinv_w * dinv_w)[v_glob[real]]
        s1[real] = (dinv_1 * dinv_1)[v_glob[real]]
        snw = np.ascontiguousarray(sw.reshape(NBLK, 128).T).astype(np.float16)
        sn1 = np.ascontiguousarray(s1.reshape(NBLK, 128).T).astype(np.float16)
        per_core.append((idxs, dlocs, nws, n1s, snw, sn1))

    return K_ts, G_s, base_pos, per_core


def _build(K_ts, G_s, base_pos):
    f32 = mybir.dt.float32
    f16 = mybir.dt.float16
    i16 = mybir.dt.int16
    nc = bacc.Bacc(None, target_bir_lowering=False, num_swdge_queues=4,
                   num_devices=NCORES)

    xT_d = nc.dram_tensor("xT", [FIN, SH], f32, kind="ExternalInput")
    idx_d = [nc.dram_tensor(f"idx{s}", [G_s[s], 128, SLOTS // 16], i16,
                            kind="ExternalInput") for s in range(NSUB)]
    dloc_d = [nc.dram_tensor(f"dloc{s}", [128, G_s[s] * CPG], f16,
                             kind="ExternalInput") for s in range(NSUB)]
    nw_d = [nc.dram_tensor(f"nw{s}", [128, G_s[s] * CPG], f16,
                           kind="ExternalInput") for s in range(NSUB)]
    n1_d = [nc.dram_tensor(f"n1{s}", [128, G_s[s] * CPG], f16,
                           kind="ExternalInput") for s in range(NSUB)]
    snw_d = nc.dram_tensor("snw", [128, NBLK], f16, kind="ExternalInput")
    sn1_d = nc.dram_tensor("sn1", [128, NBLK], f16, kind="ExternalInput")
    w1_d = nc.dram_tensor("W1", [FIN, HID], f32, kind="ExternalInput")
    w2_d = nc.dram_tensor("W2", [HID, HID], f16, kind="ExternalInput")
    wml_d = nc.dram_tensor("Wml", [HID, 2 * OUT], f16, kind="ExternalInput")
    b1_d = nc.dram_tensor("b1", [HID, 1], f32, kind="ExternalInput")
    b2_d = nc.dram_tensor("b2", [HID, 1], f32, kind="ExternalInput")
    bml_d = nc.dram_tensor("bml", [2 * OUT, 1], f32, kind="ExternalInput")
    iota_d = nc.dram_tensor("iota", [128, CPG * 128], f16, kind="ExternalInput")
    id128_d = nc.dram_tensor("id128", [128, 128], f16, kind="ExternalInput")
    id64_d = nc.dram_tensor("id64", [HID, HID], f16, kind="ExternalInput")
    out_d = nc.dram_tensor("out", [SH, 2 * OUT], f32, kind="ExternalOutput")

    ag_in = [nc.dram_tensor(f"ag_in{i}", [SH, HID], f16) for i in range(3)]
    tables = [nc.dram_tensor(f"table{i}", [NPAD, HID], f16, addr_space="Shared")
              for i in range(3)]

    with tile.TileContext(nc) as tc:
        with (
            tc.tile_pool(name="const", bufs=1) as kpool,
            tc.tile_pool(name="meta", bufs=1) as mpool,
            tc.tile_pool(name="idx", bufs=8) as ipool,
            tc.tile_pool(name="g", bufs=3) as gpool,
            tc.tile_pool(name="b", bufs=3) as bpool,
            tc.tile_pool(name="gh", bufs=3) as ghpool,
            tc.tile_pool(name="rows", bufs=2) as rpool,
            tc.tile_pool(name="selfh", bufs=1) as selfpool,
            tc.tile_pool(name="hcol", bufs=1) as hpool,
            tc.tile_pool(name="stf", bufs=1) as stpool,
            tc.tile_pool(name="xt", bufs=3) as xtpool,
            tc.tile_pool(name="pagg", bufs=4, space="PSUM") as pagg,
            tc.tile_pool(name="pproj", bufs=2, space="PSUM") as pproj,
            tc.tile_pool(name="ptr", bufs=2, space="PSUM") as ptr,
        ):
            nc.gpsimd.load_library(mlp)

            def load_const(pool, dram, shape, dt):
                t = pool.tile(shape, dt, name=dram.name + "_t")
                nc.sync.dma_start(t[:], dram[:])
                return t

            iota_t = load_const(kpool, iota_d, [128, CPG * 128], f16)
            id128_t = load_const(kpool, id128_d, [128, 128], f16)
            id64_t = load_const(kpool, id64_d, [HID, HID], f16)
            w1_t = load_const(kpool, w1_d, [FIN, HID], f32)
            w2_t = load_const(kpool, w2_d, [HID, HID], f16)
            wml_t = load_const(kpool, wml_d, [HID, 2 * OUT], f16)
            b1_t = load_const(kpool, b1_d, [HID, 1], f32)
            b2_t = load_const(kpool, b2_d, [HID, 1], f32)
            bml_t = load_const(kpool, bml_d, [2 * OUT, 1], f32)
            snw_t = load_const(kpool, snw_d, [128, NBLK], f16)
            sn1_t = load_const(kpool, sn1_d, [128, NBLK], f16)

            dloc_t, nw_t, n1_t = [], [], []
            for s in range(NSUB):
                dt_ = mpool.tile([128, G_s[s] * CPG], f16, tag=f"dl{s}")
                nc.sync.dma_start(dt_[:], dloc_d[s][:])
                dloc_t.append(dt_)
                wt = mpool.tile([128, G_s[s] * CPG], f16, tag=f"nw{s}")
                nc.sync.dma_start(wt[:], nw_d[s][:])
                nw_t.append(wt)
                ot = mpool.tile([128, G_s[s] * CPG], f16, tag=f"n1{s}")
                nc.sync.dma_start(ot[:], n1_d[s][:])
                n1_t.append(ot)

            def rows_tile():
                return rpool.tile([128, NBLK, HID], f16, tag="rows",
                                  name="rows")

            # ---- table0 = x @ W1 (fp16 rows) ----
            rows_prev = rows_tile()
            for t in range(NBLK):
                xt = xtpool.tile([128, 128], f32, tag="xt")
                nc.sync.dma_start(xt[:], xT_d[:, t * 128:(t + 1) * 128])
                pp = pproj.tile([128, HID], f32, tag="pproj")
                nc.tensor.matmul(pp[:], lhsT=xt[:], rhs=w1_t[:],
                                 start=True, stop=True)
                nc.scalar.activation(rows_prev[:, t, :], pp[:],
                                     mybir.ActivationFunctionType.Copy)
            nc.sync.dma_start(
                ag_in[0][:].rearrange("(t p) f -> p t f", p=128), rows_prev[:])

            def allgather(i):
                nc.gpsimd.collective_compute(
                    "AllGather", mybir.AluOpType.bypass,
                    replica_groups=[list(range(NCORES))],
                    ins=[ag_in[i][:]], outs=[tables[i][:]],
                )

            allgather(0)

            gq = [0]

            def gather_pass(tbl_i, norms, snorm_t, func, bias_t, rows_prev,
                            w_next):
                """One aggregation pass over tables[tbl_i].

                Returns rows_next (projected by w_next) or hcol (w_next None,
                final pass -> mu/ls columns)."""
                selfh = selfpool.tile([128, NBLK, HID], f16, tag="selfh")
                nc.vector.tensor_tensor(
                    out=selfh[:], in0=rows_prev[:],
                    in1=snorm_t[:].to_broadcast([128, NBLK, HID]),
                    op=mybir.AluOpType.mult,
                )
                hcol = hpool.tile([HID, NBLK, 128], f16, tag="hcol")
                rows_next = rows_tile() if w_next is not None else None
                cur = {}

                def ensure(s, g):
                    if s in cur and cur[s][0] == g:
                        return cur[s][1]
                    it = ipool.tile([128, SLOTS // 16], i16, tag="idx")
                    nc.sync.dma_start(it[:], idx_d[s][g])
                    gt = gpool.tile([128, CPG, HID], f32, tag=f"g{s}")
                    nc.gpsimd.dma_gather(
                        gt[:],
                        tables[tbl_i][s * SUB:(s + 1) * SUB, :]
                        .bitcast(f32)
                        .rearrange("(r two) f -> r (two f)", two=2),
                        it[:],
                        SLOTS, SLOTS, HID, queue_num=gq[0] % 4,
                    )
                    gq[0] += 1
                    gt16 = gt[:].bitcast(f16)
                    bt = bpool.tile([128, CPG, 128], f16, tag=f"b{s}")
                    nc.vector.tensor_tensor(
                        out=bt[:],
                        in0=iota_t[:].rearrange("p (j v) -> p j v", j=CPG),
                        in1=dloc_t[s][:, g * CPG:(g + 1) * CPG].to_broadcast(
                            [128, CPG, 128]),
                        op=mybir.AluOpType.is_equal,
                    )
                    gh = ghpool.tile([128, CPG, HID], f16, tag=f"gh{s}")
                    nt = norms[s]
                    nc.vector.tensor_tensor(
                        out=gh[:HALF],
                        in0=gt16[:HALF, :, 0:HID],
                        in1=nt[:HALF, g * CPG:(g + 1) * CPG].to_broadcast(
                            [HALF, CPG, HID]),
                        op=mybir.AluOpType.mult,
                    )
                    nc.vector.tensor_tensor(
                        out=gh[HALF:],
                        in0=gt16[HALF:, :, HID:2 * HID],
                        in1=nt[HALF:, g * CPG:(g + 1) * CPG].to_broadcast(
                            [HALF, CPG, HID]),
                        op=mybir.AluOpType.mult,
                    )
                    cur[s] = (g, (gh, bt))
                    return gh, bt

                for t in range(NBLK):
                    ps = pagg.tile([HID, 128], f32, tag="pagg")
                    nc.tensor.matmul(ps[:], lhsT=selfh[:, t, :], rhs=id128_t[:],
                                     start=True, stop=False)
                    nchunks = int(K_ts[t].sum())
                    ci = 0
                    for s in range(NSUB):
                        for k in range(int(K_ts[t, s])):
                            pos = int(base_pos[t, s]) + k
                            g, j = divmod(pos, CPG)
                            gh, bt = ensure(s, g)
                            ci += 1
                            nc.tensor.matmul(
                                ps[:], lhsT=gh[:, j, :], rhs=bt[:, j, :],
                                start=False, stop=(ci == nchunks),
                            )
                    nc.scalar.activation(hcol[:, t, :], ps[:], func,
                                         bias=bias_t[:])
                    if w_next is not None:
                        pp = pproj.tile([128, HID], f32, tag="pproj")
                        nc.tensor.matmul(pp[:], lhsT=hcol[:, t, :],
                                         rhs=w_next[:], start=True, stop=True)
                        nc.scalar.activation(rows_next[:, t, :], pp[:],
                                             mybir.ActivationFunctionType.Copy)
                return rows_next if w_next is not None else hcol

            relu = mybir.ActivationFunctionType.Relu
            copy = mybir.ActivationFunctionType.Identity

            # ---- layer 1: aggregate table0, relu(+b1), project W2 ----
            rows1 = gather_pass(0, nw_t, snw_t, relu, b1_t, rows_prev, w2_t)
            nc.sync.dma_start(
                ag_in[1][:].rearrange("(t p) f -> p t f", p=128), rows1[:])
            allgather(1)

            # ---- layer 2: aggregate table1, +b2, project [Wmu|Wls] ----
            rows2 = gather_pass(1, nw_t, snw_t, copy, b2_t, rows1, wml_t)
            nc.sync.dma_start(
                ag_in[2][:].rearrange("(t p) f -> p t f", p=128), rows2[:])
            allgather(2)

            # ---- layer 3: aggregate table2, +[bmu|bls] -> mu/ls columns ----
            mlcol = gather_pass(2, n1_t, sn1_t, copy, bml_t, rows2, None)

            # transpose [64, 128] -> [128, 64] per block and store
            stf = stpool.tile([128, NBLK, 2 * OUT], f32, tag="stf")
            for t in range(NBLK):
                pt = ptr.tile([128, 2 * OUT], f32, tag="ptr")
                nc.tensor.matmul(pt[:], lhsT=mlcol[:, t, :], rhs=id64_t[:],
                                 start=True, stop=True)
                nc.scalar.activation(stf[:, t, :], pt[:], copy)
            nc.sync.dma_start(
                out_d[:].rearrange("(t p) f -> p t f", p=128), stf[:])

    # Tile round-robins Pool-DMA completion sems over 8 DMASW lanes without
    # queue awareness, but each sem is hardware-locked to the first SWDGE
    # queue that increments it. Rewrite each gather's queue to lane % 4 so
    # every lane's sem is only ever incremented from one queue.
    for fn in nc.m.functions:
        for blk in fn.blocks:
            for ins in blk.instructions:
                if isinstance(ins, mybir.InstDMAGatherAnt) and ins.sync_info:
                    for u in ins.sync_info.on_update:
                        name = getattr(u, "ant_name", "") or ""
                        if name.startswith("DMASW"):
                            ins.queue_num = int(name[5:].split("_")[0]) % 4
                            break

    nc.compile()
    return nc


def _run(inputs, trace=False):
    x = np.asarray(inputs["x"], np.float32)
    K_ts, G_s, base_pos, per_core = _prep(
        x, np.asarray(inputs["edge_index"]), np.asarray(inputs["edge_weight"])
    )
    nc = _build(K_ts, G_s, base_pos)

    x_pad = np.zeros((NPAD, FIN), np.float32)
    x_pad[:N] = x
    iota = np.tile(np.arange(128, dtype=np.float16)[None, :], (128, CPG))
    shared = {
        "W1": np.asarray(inputs["W1"], np.float32),
        "W2": np.asarray(inputs["W2"], np.float16),
        "Wml": np.concatenate(
            [np.asarray(inputs["Wmu"], np.float16),
             np.asarray(inputs["Wls"], np.float16)], axis=1),
        "b1": np.asarray(inputs["b1"], np.float32).reshape(HID, 1),
        "b2": np.asarray(inputs["b2"], np.float32).reshape(HID, 1),
        "bml": np.concatenate(
            [np.asarray(inputs["bmu"], np.float32),
             np.asarray(inputs["bls"], np.float32)]).reshape(2 * OUT, 1),
        "iota": iota.reshape(128, CPG * 128),
        "id128": np.eye(128, dtype=np.float16),
        "id64": np.eye(HID, dtype=np.float16),
    }
    in_maps = []
    for d in range(NCORES):
        idxs, dlocs, nws, n1s, snw, sn1 = per_core[d]
        m = dict(shared)
        m["xT"] = np.ascontiguousarray(x_pad[d * SH:(d + 1) * SH].T)
        m["snw"] = snw
        m["sn1"] = sn1
        for s in range(NSUB):
            m[f"idx{s}"] = idxs[s]
            m[f"dloc{s}"] = dlocs[s]
            m[f"nw{s}"] = nws[s]
            m[f"n1{s}"] = n1s[s]
        in_maps.append(m)

    res = run_bass_kernel_spmd(nc, in_maps, core_ids=list(range(NCORES)), trace=trace)
    full = np.concatenate([res.results[d]["out"] for d in range(NCORES)], axis=0)
    mu = full[:N, :OUT].copy()
    logstd = full[:N, OUT:].copy()
    return (mu, logstd), res


def kernel(**inputs):
    (mu, logstd), _ = _run(inputs, trace=False)
    return mu, logstd


# revision 20
# speedup vs baseline: 1.3138x; 1.0316x over previous
"""GCN VGAE encoder (nn_Encoder_25065429139538) on 8 Trainium2 NeuronCores.

Strategy (sharding_hint: shard nodes across cores, partition edges by dst,
replicate weights):
  - Nodes padded to 100352 = 8 x 12544; core d owns dst rows [d*SH, (d+1)*SH).
  - Per-edge GCN normalization (incl. self-loops) folded into one per-edge
    scalar on the host; duplicate (src,dst) merged.
  - The per-layer gather tables are fp16, stored partition-major per
    shard-half ([rank, p, t, f] = [1024, 49, 64] per half) so the staging
    store and AllGather inputs are fully contiguous. dma_gather reads them
    as 256B pair-rows through an f32-bitcast view (the fast 64-elem ucode
    path); edges are parity-sorted inside each 128-slot chunk (slots 0-63
    use cols 0:64 of their pair-row, slots 64-127 use cols 64:128), making
    the half-select two static DVE slices.
  - Aggregation per dst 128-block via one-hot matmul accumulating in PSUM
    [HID, 128]: lhsT = scaled gathered rows (fp16), rhs = one-hot built from
    dst-local ids by DVE is_equal. Self-loops ride as one extra matmul with
    rhs = identity and lhsT = previous-layer rows scaled by dinv^2.
  - Bias (+ReLU layer 1) applied by ScalarE during PSUM eviction; the next
    layer's projection (W2 / [Wmu|Wls]) is folded BEFORE the AllGather.
  - Each AllGather is split into two shard-half collectives fired
    back-to-back at the pass boundary; a prefetch burst of stream-0 gathers
    is emitted between them so the gather pump restarts during the second
    collective.
  - All gather metadata is host-transposed partition-major and SBUF-resident.
"""

import math

import numpy as np

import concourse.bass as bass
import concourse.bacc as bacc
import concourse.mybir as mybir
import concourse.tile as tile
from concourse.bass_utils import run_bass_kernel_spmd
from concourse.library_config import mlp

# ---- problem constants (hardcoded per contract) ----
N = 100000
FIN, HID, OUT = 128, 64, 32
NCORES = 8

# ---- layout constants ----
SH = 12544             # rows per core (100352 / 8)
NPAD = SH * NCORES     # 100352
NBLK = SH // 128       # 98 dst blocks per core
NSUB = 2               # src subtables = shard halves (int16 pair-rows < 25088)
HBLK = 49              # blocks per shard half
SUB = NPAD // NSUB     # 50176 nodes per subtable
SUBP = SUB // 2        # 25088 pair-rows per subtable
SLOTS = 1024           # gather slots per dma_gather instruction
CPG = SLOTS // 128     # chunks per gather group = 8
HALF = 64              # parity slots per chunk
NBURST = 6             # stream-0 gather groups prefetched across the AG wait


def _wrap_idx(slots_i16):
    """[G*SLOTS] int16 -> [G, 128, SLOTS//16]: slot i at [i%16 (+16m), i//16]."""
    g = slots_i16.reshape(-1, SLOTS // 16, 16)          # [G, S/16, 16]
    g = np.swapaxes(g, 1, 2)                            # [G, 16, S/16]
    return np.tile(g, (1, 8, 1)).astype(np.int16)       # [G, 128, S/16]


def _prep(x, edge_index, edge_weight):
    """Host-side edge partitioning. Returns (structure, per-core arrays)."""
    src = np.asarray(edge_index[0], dtype=np.int64)
    dst = np.asarray(edge_index[1], dtype=np.int64)
    ew = np.asarray(edge_weight, dtype=np.float32)

    deg_w = np.zeros(N, np.float32)
    np.add.at(deg_w, dst, ew)
    deg_w += 1.0  # self-loop weight
    deg_1 = (np.bincount(dst, minlength=N) + 1).astype(np.float32)
    dinv_w = 1.0 / np.sqrt(deg_w)
    dinv_1 = 1.0 / np.sqrt(deg_1)

    nw = dinv_w[src] * ew * dinv_w[dst]
    n1 = dinv_1[src] * dinv_1[dst]

    # merge duplicate (src, dst) pairs (self-loops handled separately)
    key = src * NPAD + dst
    ukey, inv = np.unique(key, return_inverse=True)
    unw = np.zeros(len(ukey), np.float32)
    un1 = np.zeros(len(ukey), np.float32)
    np.add.at(unw, inv, nw)
    np.add.at(un1, inv, n1)
    usrc = ukey // NPAD
    udst = ukey % NPAD

    core = udst // SH
    t_all = (udst % SH) // 128
    dloc_all = (udst % SH) % 128

    # src position in the partition-major half-tables:
    # half s, row loc = (rank*128 + p)*HBLK + t_within_half
    r_s = usrc // SH
    off = usrc % SH
    p_s = off % 128
    tb = off // 128
    s_all = tb // HBLK
    loc_all = (r_s * 128 + p_s) * HBLK + (tb % HBLK)
    prow_all = loc_all // 2
    par_all = loc_all % 2

    # per (core, t, s, par) counts -> shared chunk structure K_ts
    cell = ((core * NBLK + t_all) * NSUB + s_all) * 2 + par_all
    cnt = np.bincount(cell, minlength=NCORES * NBLK * NSUB * 2).reshape(
        NCORES, NBLK, NSUB, 2
    )
    K_ts = np.maximum(1, np.ceil(cnt.max(axis=0).max(axis=-1) / HALF).astype(np.int64))
    C_s = K_ts.sum(axis=0)                      # chunks per s-stream
    G_s = [int(math.ceil(int(c) / CPG)) for c in C_s]
    base_pos = np.zeros((NBLK, NSUB), np.int64)  # chunk stream position of (t,s)
    for s in range(NSUB):
        base_pos[:, s] = np.cumsum(np.concatenate([[0], K_ts[:-1, s]]))

    per_core = []
    # order by (core, t, s, par, prow)
    order = np.lexsort((prow_all, par_all, s_all, t_all, core))
    oprow, opar, os_, ot, odloc, onw, on1, ocore = (
        prow_all[order], par_all[order], s_all[order], t_all[order],
        dloc_all[order], unw[order], un1[order], core[order],
    )
    cstart = np.searchsorted(ocore, np.arange(NCORES + 1))
    for d in range(NCORES):
        lo, hi = cstart[d], cstart[d + 1]
        dt, ds, dp = ot[lo:hi], os_[lo:hi], opar[lo:hi]
        dpr, ddl = oprow[lo:hi], odloc[lo:hi]
        dnw, dn1 = onw[lo:hi], on1[lo:hi]
        cell_d = (dt * NSUB + ds) * 2 + dp
        cello = np.searchsorted(cell_d, np.arange(NBLK * NSUB * 2 + 1))
        idxs, dlocs, nws, n1s = [], [], [], []
        for s in range(NSUB):
            nslot = G_s[s] * SLOTS
            sl = np.zeros(nslot, np.int64)
            dl = np.zeros(nslot, np.float32)
            wv = np.zeros(nslot, np.float32)
            v1 = np.zeros(nslot, np.float32)
            for t in range(NBLK):
                p0 = base_pos[t, s]
                for par in range(2):
                    c = (t * NSUB + s) * 2 + par
                    a, b = cello[c], cello[c + 1]
                    n = b - a
                    for k in range(int(math.ceil(n / HALF)) or 0):
                        u, v = k * HALF, min((k + 1) * HALF, n)
                        base = (p0 + k) * 128 + par * HALF
                        sl[base:base + v - u] = dpr[a + u:a + v]
                        dl[base:base + v - u] = ddl[a + u:a + v]
                        wv[base:base + v - u] = dnw[a + u:a + v]
                        v1[base:base + v - u] = dn1[a + u:a + v]
            idxs.append(_wrap_idx(sl.astype(np.int16)))
            dlocs.append(np.ascontiguousarray(
                dl.reshape(G_s[s], CPG, 128).transpose(2, 0, 1)
            ).astype(np.float16).reshape(128, -1))
            nws.append(np.ascontiguousarray(
                wv.reshape(G_s[s], CPG, 128).transpose(2, 0, 1)
            ).astype(np.float16).reshape(128, -1))
            n1s.append(np.ascontiguousarray(
                v1.reshape(G_s[s], CPG, 128).transpose(2, 0, 1)
            ).astype(np.float16).reshape(128, -1))
        # self-loop norms [128, NBLK]: [p, t] = dinv^2 of row d*SH + t*128 + p
        v_glob = np.arange(d * SH, (d + 1) * SH, dtype=np.int64)
        real = v_glob < N
        sw = np.zeros(SH, np.float32)
        s1 = np.zeros(SH, np.float32)
        sw[real] = (dinv_w * dinv_w)[v_glob[real]]
        s1[real] = (dinv_1 * dinv_1)[v_glob[real]]
        snw = np.ascontiguousarray(sw.reshape(NBLK, 128).T).astype(np.float16)
        sn1 = np.ascontiguousarray(s1.reshape(NBLK, 128).T).astype(np.float16)
        per_core.append((idxs, dlocs, nws, n1s, snw, sn1))

    return K_ts, G_s, base_pos, per_core


def _build(K_ts, G_s, base_pos):
    f32 = mybir.dt.float32
    f16 = mybir.dt.float16
    i16 = mybir.dt.int16
    nc = bacc.Bacc(None, target_bir_lowering=False, num_swdge_queues=4,
                   num_devices=NCORES)

    xT_d = nc.dram_tensor("xT", [FIN, SH], f32, kind="ExternalInput")
    idx_d = [nc.dram_tensor(f"idx{s}", [G_s[s], 128, SLOTS // 16], i16,
                            kind="ExternalInput") for s in range(NSUB)]
    dloc_d = [nc.dram_tensor(f"dloc{s}", [128, G_s[s] * CPG], f16,
                             kind="ExternalInput") for s in range(NSUB)]
    nw_d = [nc.dram_tensor(f"nw{s}", [128, G_s[s] * CPG], f16,
                           kind="ExternalInput") for s in range(NSUB)]
    n1_d = [nc.dram_tensor(f"n1{s}", [128, G_s[s] * CPG], f16,
                           kind="ExternalInput") for s in range(NSUB)]
    snw_d = nc.dram_tensor("snw", [128, NBLK], f16, kind="ExternalInput")
    sn1_d = nc.dram_tensor("sn1", [128, NBLK], f16, kind="ExternalInput")
    w1_d = nc.dram_tensor("W1", [FIN, HID], f32, kind="ExternalInput")
    w2_d = nc.dram_tensor("W2", [HID, HID], f16, kind="ExternalInput")
    wml_d = nc.dram_tensor("Wml", [HID, 2 * OUT], f16, kind="ExternalInput")
    b1_d = nc.dram_tensor("b1", [HID, 1], f32, kind="ExternalInput")
    b2_d = nc.dram_tensor("b2", [HID, 1], f32, kind="ExternalInput")
    bml_d = nc.dram_tensor("bml", [2 * OUT, 1], f32, kind="ExternalInput")
    iota_d = nc.dram_tensor("iota", [128, CPG * 128], f16, kind="ExternalInput")
    id128_d = nc.dram_tensor("id128", [128, 128], f16, kind="ExternalInput")
    id64_d = nc.dram_tensor("id64", [HID, HID], f16, kind="ExternalInput")
    out_d = nc.dram_tensor("out", [128, NBLK, 2 * OUT], f32,
                           kind="ExternalOutput")

    ag_in = [[nc.dram_tensor(f"ag_in{i}_{h}", [128, HBLK, HID], f16)
              for h in range(2)] for i in range(3)]
    tables = [[nc.dram_tensor(f"table{i}_{h}", [NCORES * 128, HBLK, HID], f16,
                              addr_space="Shared") for h in range(2)]
              for i in range(3)]

    with tile.TileContext(nc) as tc:
        with (
            tc.tile_pool(name="const", bufs=1) as kpool,
            tc.tile_pool(name="meta", bufs=1) as mpool,
            tc.tile_pool(name="idx", bufs=10) as ipool,
            tc.tile_pool(name="g", bufs=3) as gpool,
            tc.tile_pool(name="b", bufs=3) as bpool,
            tc.tile_pool(name="gh", bufs=3) as ghpool,
            tc.tile_pool(name="g0", bufs=NBURST) as gpool0,
            tc.tile_pool(name="b0", bufs=NBURST) as bpool0,
            tc.tile_pool(name="gh0", bufs=NBURST) as ghpool0,
            tc.tile_pool(name="rows", bufs=2) as rpool,
            tc.tile_pool(name="selfh", bufs=1) as selfpool,
            tc.tile_pool(name="hcol", bufs=1) as hpool,
            tc.tile_pool(name="stf", bufs=1) as stpool,
            tc.tile_pool(name="xt", bufs=6) as xtpool,
            tc.tile_pool(name="pagg", bufs=4, space="PSUM") as pagg,
            tc.tile_pool(name="pproj", bufs=2, space="PSUM") as pproj,
            tc.tile_pool(name="ptr", bufs=2, space="PSUM") as ptr,
        ):
            nc.gpsimd.load_library(mlp)

            def load_const(pool, dram, shape, dt):
                t = pool.tile(shape, dt, name=dram.name + "_t")
                nc.sync.dma_start(t[:], dram[:])
                return t

            iota_t = load_const(kpool, iota_d, [128, CPG * 128], f16)
            id128_t = load_const(kpool, id128_d, [128, 128], f16)
            id64_t = load_const(kpool, id64_d, [HID, HID], f16)
            w1_t = load_const(kpool, w1_d, [FIN, HID], f32)
            w2_t = load_const(kpool, w2_d, [HID, HID], f16)
            wml_t = load_const(kpool, wml_d, [HID, 2 * OUT], f16)
            b1_t = load_const(kpool, b1_d, [HID, 1], f32)
            b2_t = load_const(kpool, b2_d, [HID, 1], f32)
            bml_t = load_const(kpool, bml_d, [2 * OUT, 1], f32)
            snw_t = load_const(kpool, snw_d, [128, NBLK], f16)
            sn1_t = load_const(kpool, sn1_d, [128, NBLK], f16)

            dloc_t, nw_t, n1_t = [], [], []
            for s in range(NSUB):
                dt_ = mpool.tile([128, G_s[s] * CPG], f16, tag=f"dl{s}",
                                 name="dl")
                nc.sync.dma_start(dt_[:], dloc_d[s][:])
                dloc_t.append(dt_)
                wt = mpool.tile([128, G_s[s] * CPG], f16, tag=f"nw{s}",
                                name="nw")
                nc.sync.dma_start(wt[:], nw_d[s][:])
                nw_t.append(wt)
                ot = mpool.tile([128, G_s[s] * CPG], f16, tag=f"n1{s}",
                                name="n1")
                nc.sync.dma_start(ot[:], n1_d[s][:])
                n1_t.append(ot)

            def rows_tile():
                return rpool.tile([128, NBLK, HID], f16, tag="rows",
                                  name="rows")

            def allgather(i, h):
                nc.gpsimd.collective_compute(
                    "AllGather", mybir.AluOpType.bypass,
                    replica_groups=[list(range(NCORES))],
                    ins=[ag_in[i][h][:]], outs=[tables[i][h][:]],
                )

            def store_half(i, rows, h):
                nc.sync.dma_start(ag_in[i][h][:],
                                  rows[:, h * HBLK:(h + 1) * HBLK, :])

            # ---- table0 = x @ W1 (fp16 rows); AG-a fired mid-stage ----
            rows_prev = rows_tile()
            for t in range(NBLK):
                xt = xtpool.tile([128, 128], f32, tag="xt", name="xt")
                nc.sync.dma_start(xt[:], xT_d[:, t * 128:(t + 1) * 128])
                pp = pproj.tile([128, HID], f32, tag="pproj", name="pp")
                nc.tensor.matmul(pp[:], lhsT=xt[:], rhs=w1_t[:],
                                 start=True, stop=True)
                nc.scalar.activation(rows_prev[:, t, :], pp[:],
                                     mybir.ActivationFunctionType.Copy)
                if t == HBLK - 1:
                    store_half(0, rows_prev, 0)
                    allgather(0, 0)
            store_half(0, rows_prev, 1)

            gq = [0]

            def gather_pass(tbl_i, norms, snorm_t, func, bias_t, rows_prev,
                            w_next, pre_ag=None):
                """One aggregation pass over tables[tbl_i][0..1].

                pre_ag (if given) is emitted after the stream-0 prefetch
                burst, so its collective overlaps the restarted pump."""
                selfh = selfpool.tile([128, NBLK, HID], f16, tag="selfh",
                                      name="selfh")
                nc.vector.tensor_tensor(
                    out=selfh[:], in0=rows_prev[:],
                    in1=snorm_t[:].to_broadcast([128, NBLK, HID]),
                    op=mybir.AluOpType.mult,
                )
                hcol = hpool.tile([HID, NBLK, 128], f16, tag="hcol",
                                  name="hcol")
                rows_next = rows_tile() if w_next is not None else None
                cur = {}
                pre = {}

                def make_group(s, g):
                    gp = gpool0 if s == 0 else gpool
                    bp = bpool0 if s == 0 else bpool
                    ghp = ghpool0 if s == 0 else ghpool
                    it = ipool.tile([128, SLOTS // 16], i16, tag="idx",
                                    name="it")
                    nc.sync.dma_start(it[:], idx_d[s][g])
                    gt = gp.tile([128, CPG, HID], f32, tag=f"g{s}", name="gt")
                    nc.gpsimd.dma_gather(
                        gt[:],
                        tables[tbl_i][s][:]
                        .rearrange("q t f -> (q t) f")
                        .bitcast(f32)
                        .rearrange("(r two) f -> r (two f)", two=2),
                        it[:],
                        SLOTS, SLOTS, HID, queue_num=gq[0] % 4,
                    )
                    gq[0] += 1
                    gt16 = gt[:].bitcast(f16)
                    bt = bp.tile([128, CPG, 128], f16, tag=f"b{s}", name="bt")
                    nc.vector.tensor_tensor(
                        out=bt[:],
                        in0=iota_t[:].rearrange("p (j v) -> p j v", j=CPG),
                        in1=dloc_t[s][:, g * CPG:(g + 1) * CPG].to_broadcast(
                            [128, CPG, 128]),
                        op=mybir.AluOpType.is_equal,
                    )
                    gh = ghp.tile([128, CPG, HID], f16, tag=f"gh{s}",
                                  name="gh")
                    nt = norms[s]
                    nc.vector.tensor_tensor(
                        out=gh[:HALF],
                        in0=gt16[:HALF, :, 0:HID],
                        in1=nt[:HALF, g * CPG:(g + 1) * CPG].to_broadcast(
                            [HALF, CPG, HID]),
                        op=mybir.AluOpType.mult,
                    )
                    nc.vector.tensor_tensor(
                        out=gh[HALF:],
                        in0=gt16[HALF:, :, HID:2 * HID],
                        in1=nt[HALF:, g * CPG:(g + 1) * CPG].to_broadcast(
                            [HALF, CPG, HID]),
                        op=mybir.AluOpType.mult,
                    )
                    return gh, bt

                def ensure(s, g):
                    if s in cur and cur[s][0] == g:
                        return cur[s][1]
                    if (s, g) in pre:
                        cur[s] = (g, pre.pop((s, g)))
                    else:
                        cur[s] = (g, make_group(s, g))
                    return cur[s][1]

                # prefetch burst: restart the stream-0 pump, then let the
                # second half's collective run underneath it
                for g in range(min(NBURST, G_s[0])):
                    pre[(0, g)] = make_group(0, g)
                if pre_ag is not None:
                    pre_ag()

                for t in range(NBLK):
                    ps = pagg.tile([HID, 128], f32, tag="pagg", name="ps")
                    nc.tensor.matmul(ps[:], lhsT=selfh[:, t, :], rhs=id128_t[:],
                                     start=True, stop=False)
                    nchunks = int(K_ts[t].sum())
                    ci = 0
                    for s in range(NSUB):
                        for k in range(int(K_ts[t, s])):
                            pos = int(base_pos[t, s]) + k
                            g, j = divmod(pos, CPG)
                            gh, bt = ensure(s, g)
                            ci += 1
                            nc.tensor.matmul(
                                ps[:], lhsT=gh[:, j, :], rhs=bt[:, j, :],
                                start=False, stop=(ci == nchunks),
                            )
                    nc.scalar.activation(hcol[:, t, :], ps[:], func,
                                         bias=bias_t[:])
                    if w_next is not None:
                        pp = pproj.tile([128, HID], f32, tag="pproj",
                                        name="pp")
                        nc.tensor.matmul(pp[:], lhsT=hcol[:, t, :],
                                         rhs=w_next[:], start=True, stop=True)
                        nc.scalar.activation(rows_next[:, t, :], pp[:],
                                             mybir.ActivationFunctionType.Copy)
                    else:
                        pt = ptr.tile([128, 2 * OUT], f32, tag="ptr",
                                      name="pt")
                        nc.tensor.matmul(pt[:], lhsT=hcol[:, t, :],
                                         rhs=id64_t[:], start=True, stop=True)
                        nc.scalar.activation(stf[:, t, :], pt[:],
                                             mybir.ActivationFunctionType.Copy)
                return rows_next

            relu = mybir.ActivationFunctionType.Relu
            ident = mybir.ActivationFunctionType.Identity
            stf = stpool.tile([128, NBLK, 2 * OUT], f32, tag="stf", name="stf")

            # ---- layer 1: aggregate table0, relu(+b1), project W2 ----
            rows1 = gather_pass(0, nw_t, snw_t, relu, b1_t, rows_prev, w2_t,
                                pre_ag=lambda: allgather(0, 1))
            store_half(1, rows1, 0)
            allgather(1, 0)
            store_half(1, rows1, 1)

            # ---- layer 2: aggregate table1, +b2, project [Wmu|Wls] ----
            rows2 = gather_pass(1, nw_t, snw_t, ident, b2_t, rows1, wml_t,
                                pre_ag=lambda: allgather(1, 1))
            store_half(2, rows2, 0)
            allgather(2, 0)
            store_half(2, rows2, 1)

            # ---- layer 3: aggregate table2, +[bmu|bls] -> mu/ls, store ----
            gather_pass(2, n1_t, sn1_t, ident, bml_t, rows2, None,
                        pre_ag=lambda: allgather(2, 1))
            nc.sync.dma_start(out_d[:], stf[:])

    # Tile round-robins Pool-DMA completion sems over 8 DMASW lanes without
    # queue awareness, but each sem is hardware-locked to the first SWDGE
    # queue that increments it. Rewrite each gather's queue to lane % 4 so
    # every lane's sem is only ever incremented from one queue.
    for fn in nc.m.functions:
        for blk in fn.blocks:
            for ins in blk.instructions:
                if isinstance(ins, mybir.InstDMAGatherAnt) and ins.sync_info:
                    for u in ins.sync_info.on_update:
                        name = getattr(u, "ant_name", "") or ""
                        if name.startswith("DMASW"):
                            ins.queue_num = int(name[5:].split("_")[0]) % 4
                            break

    nc.compile()
    return nc


def _run(inputs, trace=False):
    x = np.asarray(inputs["x"], np.float32)
    K_ts, G_s, base_pos, per_core = _prep(
        x, np.asarray(inputs["edge_index"]), np.asarray(inputs["edge_weight"])
    )
    nc = _build(K_ts, G_s, base_pos)

    x_pad = np.zeros((NPAD, FIN), np.float32)
    x_pad[:N] = x
    iota = np.tile(np.arange(128, dtype=np.float16)[None, :], (128, CPG))
    shared = {
        "W1": np.asarray(inputs["W1"], np.float32),
        "W2": np.asarray(inputs["W2"], np.float16),
        "Wml": np.concatenate(
            [np.asarray(inputs["Wmu"], np.float16),
             np.asarray(inputs["Wls"], np.float16)], axis=1),
        "b1": np.asarray(inputs["b1"], np.float32).reshape(HID, 1),
        "b2": np.asarray(inputs["b2"], np.float32).reshape(HID, 1),
        "bml": np.concatenate(
            [np.asarray(inputs["bmu"], np.float32),
             np.asarray(inputs["bls"], np.float32)]).reshape(2 * OUT, 1),
        "iota": iota.reshape(128, CPG * 128),
        "id128": np.eye(128, dtype=np.float16),
        "id64": np.eye(HID, dtype=np.float16),
    }
    in_maps = []
    for d in range(NCORES):
        idxs, dlocs, nws, n1s, snw, sn1 = per_core[d]
        m = dict(shared)
        m["xT"] = np.ascontiguousarray(x_pad[d * SH:(d + 1) * SH].T)
        m["snw"] = snw
        m["sn1"] = sn1
        for s in range(NSUB):
            m[f"idx{s}"] = idxs[s]
            m[f"dloc{s}"] = dlocs[s]
            m[f"nw{s}"] = nws[s]
            m[f"n1{s}"] = n1s[s]
        in_maps.append(m)

    res = run_bass_kernel_spmd(nc, in_maps, core_ids=list(range(NCORES)), trace=trace)
    full = np.concatenate(
        [res.results[d]["out"].transpose(1, 0, 2).reshape(SH, 2 * OUT)
         for d in range(NCORES)], axis=0)
    mu = full[:N, :OUT].copy()
    logstd = full[:N, OUT:].copy()
    return (mu, logstd), res


def kernel(**inputs):
    (mu, logstd), _ = _run(inputs, trace=False)
    return mu, logstd
